# revision 1
# baseline (speedup 1.0000x reference)
"""Trainium2 Bass kernel for nn_CrossAttn (linear cross-attention, B=8 N=4096 C=1024 H=16).

Strategy:
  - Data-parallel over B across the 8 NeuronCores (batch-local math, no collectives).
  - Host pre-transposes activations to x^T [C, N] (C on partitions) and casts matmul
    operands to bf16; PSUM accumulation is fp32.
  - Self stage per stream: qkv GEMMs from x^T; linear ctx = softmax_d(k^T v * scale)
    accumulated as paired v^T k matmuls (2 heads -> one 128x128 MM); softmax along the
    free axis; ctx transposed via PE into a block-diagonal 2-head tile so the output
    product q @ ctx becomes (ctx_bd)^T @ q^T with K=128, N=512.
  - Cross stage: q is the post-self activation itself (already resident transposed);
    kv GEMMs with Wkv1/Wkv2; o1 = ctx2-product + x1', o2 = ctx1-product + x2'.
  - Outputs written transposed [C, N] fp32; host un-transposes.
"""

import os
import sys

sys.path.insert(0, "/opt/trn_rl_repo")

import numpy as np
import ml_dtypes

import concourse.bass as bass
import concourse.mybir as mybir
import concourse.tile as tile
from concourse import bacc
from concourse.masks import make_identity
from concourse.bass_utils import run_bass_kernel_spmd

B, N, C, H = 8, 4096, 1024, 16
D = C // H                 # 64
SCALE = D ** -0.5          # 0.125
P = 128                    # partitions
KT = C // P                # 8 contraction tiles
NT = N // P                # 32 n-tiles (ctx accumulation)
CH = N // 512              # 8 n-chunks of 512
PAIRS = H // 2             # 8 head pairs
F32 = mybir.dt.float32
BF16 = mybir.dt.bfloat16

_CACHE = {}


def _build():
    nc = bacc.Bacc(None, target_bir_lowering=False)

    x1T_d = nc.dram_tensor("x1T", [C, N], BF16, kind="ExternalInput")
    x2T_d = nc.dram_tensor("x2T", [C, N], BF16, kind="ExternalInput")
    Wsqkv_d = nc.dram_tensor("Wsqkv", [C, 3 * C], BF16, kind="ExternalInput")
    Wkv1_d = nc.dram_tensor("Wkv1", [C, 2 * C], BF16, kind="ExternalInput")
    Wkv2_d = nc.dram_tensor("Wkv2", [C, 2 * C], BF16, kind="ExternalInput")
    o1T_d = nc.dram_tensor("o1T", [C, N], F32, kind="ExternalOutput")
    o2T_d = nc.dram_tensor("o2T", [C, N], F32, kind="ExternalOutput")
    x1p_scr = nc.dram_tensor("x1p_scratch", [C, N], BF16, kind="Internal")

    # (kt*128 + p, n) -> [p, kt, n] view for per-partition-tile DMA
    x1T_r = x1T_d[:].rearrange("(t p) n -> p t n", p=P)
    x2T_r = x2T_d[:].rearrange("(t p) n -> p t n", p=P)
    Wsq_r = Wsqkv_d[:].rearrange("(t p) c -> p t c", p=P)
    Wkv1_r = Wkv1_d[:].rearrange("(t p) c -> p t c", p=P)
    Wkv2_r = Wkv2_d[:].rearrange("(t p) c -> p t c", p=P)
    o1T_r = o1T_d[:].rearrange("(t p) n -> p t n", p=P)
    o2T_r = o2T_d[:].rearrange("(t p) n -> p t n", p=P)
    x1p_r = x1p_scr[:].rearrange("(t p) n -> p t n", p=P)

    with tile.TileContext(nc) as tc:
        with (
            tc.tile_pool(name="xbig", bufs=2) as xbig,
            tc.tile_pool(name="wts", bufs=1) as wts,
            tc.tile_pool(name="kvsb", bufs=2) as kvsb,
            tc.tile_pool(name="qts", bufs=3) as qtsp,
            tc.tile_pool(name="ctxsb", bufs=2) as ctxsb,
            tc.tile_pool(name="ctxacc", bufs=1) as ctxaccp,
            tc.tile_pool(name="smax", bufs=2) as smaxp,
            tc.tile_pool(name="stats", bufs=4) as stats,
            tc.tile_pool(name="outst", bufs=2) as outst,
            tc.tile_pool(name="singles", bufs=1) as singles,
            tc.tile_pool(name="ps_kv", bufs=2, space="PSUM") as ps_kv,
            tc.tile_pool(name="ps_ctx", bufs=1, space="PSUM") as ps_ctx,
            tc.tile_pool(name="ps_qt", bufs=2, space="PSUM") as ps_qt,
            tc.tile_pool(name="ps_out", bufs=2, space="PSUM") as ps_out,
        ):
            ident = singles.tile([P, P], F32)
            make_identity(nc, ident)

            def ctx_accumulate(xt, W, kvcol0):
                """Accumulate per-pair ctx_rawT = v^T k over all n-tiles.

                xt: [P, KT, N] bf16 (activation transposed), W: [P, KT, wcols] bf16.
                Returns SBUF tile [P, PAIRS*128] fp32: pair p cols [128p,128p+128),
                head 2p block at rows 0:64 cols +0:64, head 2p+1 at rows 64:128
                cols +64:128 (off-diagonal blocks are garbage, never read).

                Accumulation across n-tiles happens in SBUF via DVE adds — four
                per-pair PSUM groups would share a bank, and each group's
                start=True clears has_written for the WHOLE bank, corrupting the
                other pairs' accumulation.
                """
                ctx_acc = ctxaccp.tile([P, PAIRS * P], F32, tag="ctxacc")

                def pair_mms(kv, nt):
                    ctx_ps = ps_ctx.tile([P, PAIRS * P], F32, tag="ctx")
                    for p in range(PAIRS):
                        nc.tensor.matmul(
                            ctx_ps[:, p * P:(p + 1) * P],
                            lhsT=kv[:, C + p * P: C + (p + 1) * P],   # v pair
                            rhs=kv[:, p * P:(p + 1) * P],             # k pair
                            start=True, stop=True,
                        )
                    if nt == 0:
                        nc.vector.tensor_copy(ctx_acc, ctx_ps)
                    else:
                        nc.vector.tensor_add(ctx_acc, ctx_acc, ctx_ps)

                prev = None
                for nt in range(NT):
                    kv = kvsb.tile([P, 2 * C], BF16, tag="kv")
                    for ch in range(4):
                        kv_ps = ps_kv.tile([P, 512], F32, tag="kvps")
                        for kt in range(KT):
                            nc.tensor.matmul(
                                kv_ps,
                                lhsT=xt[:, kt, nt * P:(nt + 1) * P],
                                rhs=W[:, kt, kvcol0 + ch * 512: kvcol0 + (ch + 1) * 512],
                                start=(kt == 0), stop=(kt == KT - 1),
                            )
                        nc.vector.tensor_copy(kv[:, ch * 512:(ch + 1) * 512], kv_ps)
                    if prev is not None:
                        pair_mms(*prev)
                    prev = (kv, nt)
                pair_mms(*prev)
                return ctx_acc

            def softmax_pair(ctx_ps, p, ctx_bd):
                """Softmax over d (free axis) of the two diag blocks of pair p, then
                PE-transpose into slice p of the block-diagonal bf16 ctx tile."""
                S = smaxp.tile([P, P], F32, tag="smax")
                nc.vector.memset(S, 0.0)
                for r0 in (0, 64):
                    blk = ctx_ps[r0:r0 + 64, p * P + r0: p * P + r0 + 64]
                    mx = stats.tile([P, 1], F32, tag="mx")
                    nc.vector.reduce_max(mx[r0:r0 + 64], blk, axis=mybir.AxisListType.X)
                    ng = stats.tile([P, 1], F32, tag="ng")
                    nc.scalar.mul(ng[r0:r0 + 64], mx[r0:r0 + 64], -SCALE)
                    se = stats.tile([P, 1], F32, tag="se")
                    nc.scalar.activation(
                        S[r0:r0 + 64, r0:r0 + 64], blk,
                        mybir.ActivationFunctionType.Exp,
                        bias=ng[r0:r0 + 64], scale=SCALE,
                        accum_out=se[r0:r0 + 64],
                    )
                    rv = stats.tile([P, 1], F32, tag="rv")
                    nc.vector.reciprocal(rv[r0:r0 + 64], se[r0:r0 + 64])
                    nc.vector.tensor_scalar_mul(
                        S[r0:r0 + 64, r0:r0 + 64], S[r0:r0 + 64, r0:r0 + 64],
                        rv[r0:r0 + 64],
                    )
                tr_ps = ps_out.tile([P, P], F32, tag="psout")
                nc.tensor.transpose(tr_ps, S, ident)
                nc.vector.tensor_copy(ctx_bd[:, p, :], tr_ps)

            def self_stage(xt, W, xp_out):
                """One self-attention branch: returns nothing; writes x' (bf16,
                transposed) into xp_out [P, KT, N]."""
                ctx_ps = ctx_accumulate(xt, W, kvcol0=C)
                ctx_bd = ctxsb.tile([P, PAIRS, P], BF16, tag="ctx_bd")
                for p in range(PAIRS):
                    softmax_pair(ctx_ps, p, ctx_bd)
                    for ch in range(CH):
                        qt_ps = ps_qt.tile([P, 512], F32, tag="qt")
                        for kt in range(KT):
                            nc.tensor.matmul(
                                qt_ps,
                                lhsT=W[:, kt, p * P:(p + 1) * P],
                                rhs=xt[:, kt, ch * 512:(ch + 1) * 512],
                                start=(kt == 0), stop=(kt == KT - 1),
                            )
                        qts = qtsp.tile([P, 512], BF16, tag="qts")
                        nc.vector.tensor_copy(qts, qt_ps)
                        out_ps = ps_out.tile([P, 512], F32, tag="psout")
                        nc.tensor.matmul(out_ps, lhsT=ctx_bd[:, p, :], rhs=qts,
                                         start=True, stop=True)
                        nc.vector.tensor_add(
                            xp_out[:, p, ch * 512:(ch + 1) * 512],
                            out_ps, xt[:, p, ch * 512:(ch + 1) * 512],
                        )

            def cross_out(o_r, ctx_bd, qpt):
                """o = merge(q @ ctx) + q_stream_residual, written transposed fp32 to DRAM."""
                for p in range(PAIRS):
                    for ch in range(CH):
                        out_ps = ps_out.tile([P, 512], F32, tag="psout")
                        nc.tensor.matmul(out_ps, lhsT=ctx_bd[:, p, :],
                                         rhs=qpt[:, p, ch * 512:(ch + 1) * 512],
                                         start=True, stop=True)
                        stg = outst.tile([P, 512], F32, tag="stg")
                        nc.vector.tensor_add(stg, out_ps,
                                             qpt[:, p, ch * 512:(ch + 1) * 512])
                        nc.sync.dma_start(
                            out=o_r[:, p, ch * 512:(ch + 1) * 512], in_=stg)

            # ---- self stage, stream 1 ----
            x1t = xbig.tile([P, KT, N], BF16, tag="xbig")
            nc.sync.dma_start(out=x1t, in_=x1T_r)
            Wsq = wts.tile([P, KT, 3 * C], BF16, tag="wts")
            nc.sync.dma_start(out=Wsq, in_=Wsq_r)
            x1p = xbig.tile([P, KT, N], BF16, tag="xbig")
            self_stage(x1t, Wsq, x1p)
            nc.sync.dma_start(out=x1p_r, in_=x1p)     # spill for later reload

            # ---- self stage, stream 2 ----
            x2t = xbig.tile([P, KT, N], BF16, tag="xbig")   # reuses x1t slot
            nc.sync.dma_start(out=x2t, in_=x2T_r)
            x2p = xbig.tile([P, KT, N], BF16, tag="xbig")   # reuses x1p slot
            self_stage(x2t, Wsq, x2p)

            # ---- cross stage ----
            Wkv2 = wts.tile([P, KT, 2 * C], BF16, tag="wts")
            nc.sync.dma_start(out=Wkv2, in_=Wkv2_r)
            ctx2_ps = ctx_accumulate(x2p, Wkv2, kvcol0=0)
            ctx2_bd = ctxsb.tile([P, PAIRS, P], BF16, tag="ctx_bd")
            for p in range(PAIRS):
                softmax_pair(ctx2_ps, p, ctx2_bd)

            x1pr = xbig.tile([P, KT, N], BF16, tag="xbig")  # reuses x2t slot
            nc.sync.dma_start(out=x1pr, in_=x1p_r)
            cross_out(o1T_r, ctx2_bd, x1pr)                 # o1 = q1 @ ctx2 + x1'

            Wkv1 = wts.tile([P, KT, 2 * C], BF16, tag="wts")
            nc.sync.dma_start(out=Wkv1, in_=Wkv1_r)
            ctx1_ps = ctx_accumulate(x1pr, Wkv1, kvcol0=0)
            ctx1_bd = ctxsb.tile([P, PAIRS, P], BF16, tag="ctx_bd")
            for p in range(PAIRS):
                softmax_pair(ctx1_ps, p, ctx1_bd)
            cross_out(o2T_r, ctx1_bd, x2p)                  # o2 = q2 @ ctx1 + x2'

    nc.finalize()
    return nc


def _get_nc():
    if "nc" not in _CACHE:
        _CACHE["nc"] = _build()
    return _CACHE["nc"]


def kernel(x1, x2, Wsqkv1, Wkv1, Wkv2, num_heads=16, selfattn=1, **_unused):
    x1 = np.asarray(x1, dtype=np.float32)
    x2 = np.asarray(x2, dtype=np.float32)
    Wsq_b = np.ascontiguousarray(np.asarray(Wsqkv1, np.float32)).astype(ml_dtypes.bfloat16)
    Wkv1_b = np.ascontiguousarray(np.asarray(Wkv1, np.float32)).astype(ml_dtypes.bfloat16)
    Wkv2_b = np.ascontiguousarray(np.asarray(Wkv2, np.float32)).astype(ml_dtypes.bfloat16)

    nc = _get_nc()
    in_maps = []
    for b in range(B):
        in_maps.append({
            "x1T": np.ascontiguousarray(x1[b].T).astype(ml_dtypes.bfloat16),
            "x2T": np.ascontiguousarray(x2[b].T).astype(ml_dtypes.bfloat16),
            "Wsqkv": Wsq_b,
            "Wkv1": Wkv1_b,
            "Wkv2": Wkv2_b,
        })
    res = run_bass_kernel_spmd(nc, in_maps, core_ids=list(range(B)),
                               trace=bool(int(os.environ.get("KERNEL_TRACE", "0"))))
    _CACHE["last_result"] = res
    o1 = np.stack([np.asarray(res.results[b]["o1T"], np.float32).T for b in range(B)])
    o2 = np.stack([np.asarray(res.results[b]["o2T"], np.float32).T for b in range(B)])
    return o1, o2

